# revision 5
# baseline (speedup 1.0000x reference)
"""Causal single-head attention (B=4, T=4096, C=1024, H=64) on 8 trn2 cores.

Sharding: 2 cores per batch element, 2048 queries each. Window trick for
SPMD uniformity: every core sees a 4096-wide key window with its queries at
window positions [2048, 4096). Core h of batch b gets keys x[b, 0:2048*(h+1)]
right-aligned (h=0: first 2048 key columns zero, killed via a zeroed
ones-column so they contribute nothing to numerator or denominator). Both
cores run one identical program with exact causal masking in window coords.

Perf structure (v2):
  - matmul inputs in bf16 (fp32 PSUM accumulation, fp32 softmax normalize)
  - projections col-packed: two 512-column t-blocks computed concurrently on
    array column halves -> psum partitions [0:64] / [64:128]; A/B chunk
    matmuls emitted adjacently so the PE runs both streams concurrently
  - K^T/Q^T stored double-stacked on partitions; scores row-packed: two
    key tiles (j, j+4) computed concurrently on array row halves into one
    [128, 1024] psum pair, one exp (ACT) per pair
  - O matmuls row-packed too: each key tile's 128-key contraction is split
    into two 64-key halves on array row halves, accumulating into two psum
    accumulators (po_a/po_b) summed at the end -> O runs at 2 tiles/slot
  - attention software-pipelined one item so the PE never waits on ACT;
    emission interleaves projections into the attention stream
  - causal diagonal masked post-exp with GPSIMD affine_select
  - denominator via fused ones-column; finalized by po_a+po_b add, PE
    transpose + DVE reciprocal/scale
  - DMA: per-chunk 2D descriptors (cheap triggers), first tile's triggers
    spread across idle engine queues so the PE starts in ~2us
"""

import numpy as np
import ml_dtypes

import concourse.bass as bass
import concourse.bacc as bacc
import concourse.tile as tile
from concourse import mybir
from concourse.bass_utils import run_bass_kernel_spmd

B, T, C, H = 4, 4096, 1024, 64
N_CORES = 8
TQ = 2048            # queries per core
NQB = 4              # q-blocks of 512
NCH = C // 128       # 8 contraction chunks
NTB = T // 512       # 8 key t-blocks
NKT = T // 128       # 32 key tiles
F32 = mybir.dt.float32
BF16 = mybir.dt.bfloat16

_nc_cache = {}


def build_module():
    if "nc" in _nc_cache:
        return _nc_cache["nc"]
    nc = bacc.Bacc("TRN2", target_bir_lowering=False, debug=False,
                   num_devices=N_CORES)
    xk = nc.dram_tensor("xk", [C, T], BF16, kind="ExternalInput").ap()
    # weights pre-arranged on host to [128, NCH*H] (p, ch-major)
    wq = nc.dram_tensor("wq", [128, NCH * H], BF16, kind="ExternalInput").ap()
    wk = nc.dram_tensor("wk", [128, NCH * H], BF16, kind="ExternalInput").ap()
    wv = nc.dram_tensor("wv", [128, NCH * H], BF16, kind="ExternalInput").ap()
    ones_kv = nc.dram_tensor("ones_kv", [128, NKT], F32,
                             kind="ExternalInput").ap()
    ident = nc.dram_tensor("ident", [128, 128], F32, kind="ExternalInput").ap()
    ident2 = nc.dram_tensor("ident2", [128, 64], BF16,
                            kind="ExternalInput").ap()
    # [qb, p, s, h]: per-partition-contiguous rows for full-rate DMA writes;
    # host unpermutes to [TQ, H].
    out = nc.dram_tensor("out", [NQB, 128, 4, H], F32,
                         kind="ExternalOutput").ap()

    with tile.TileContext(nc) as tc:
        with (
            tc.tile_pool(name="consts", bufs=1) as consts,
            tc.tile_pool(name="xt", bufs=2) as xt_pool,
            tc.tile_pool(name="vtmp", bufs=2) as vtmp_pool,
            tc.tile_pool(name="exps", bufs=6) as exps_pool,
            tc.tile_pool(name="fin", bufs=2) as fin_pool,
            tc.tile_pool(name="ps_s", bufs=2, space="PSUM") as ps_s,
            tc.tile_pool(name="ps_o", bufs=1, space="PSUM") as ps_o,
            tc.tile_pool(name="ps_p", bufs=2, space="PSUM") as ps_p,
        ):
            xk_r = xk.rearrange("(ch p) t -> p ch t", p=128)

            # ---- weights / consts: cheap 2D triggers ----
            w_sb = {}
            wq_t = consts.tile([128, NCH * H], BF16, name="wq_sb")
            nc.sync.dma_start(wq_t[:], wq)
            w_sb["wq"] = wq_t

            # first x tile (tb 4,5 = this core's queries): per-chunk 2D DMAs
            # spread across engine queues so transfers start immediately.
            xt_tiles = {}
            xt4 = xt_pool.tile([128, NCH, 1024], BF16, tag="xt4", name="xt4")
            trig4 = [nc.scalar, nc.gpsimd, nc.sync, nc.scalar,
                     nc.gpsimd, nc.sync, nc.scalar, nc.gpsimd]
            for ch in range(NCH):
                trig4[ch].dma_start(xt4[:, ch, :],
                                    xk_r[:, ch, 4 * 512:6 * 512])
            xt_tiles[4] = xt4

            wk_t = consts.tile([128, NCH * H], BF16, name="wk_sb")
            nc.sync.dma_start(wk_t[:], wk)
            w_sb["wk"] = wk_t
            wv_t = consts.tile([128, NCH * H], BF16, name="wv_sb")
            nc.sync.dma_start(wv_t[:], wv)
            w_sb["wv"] = wv_t

            # remaining x tiles: per-chunk 2D DMAs on sync queue
            for tb0 in (0, 2, 6):
                xt = xt_pool.tile([128, NCH, 1024], BF16, tag=f"xt{tb0}",
                                  name=f"xt{tb0}")
                for ch in range(NCH):
                    nc.sync.dma_start(
                        xt[:, ch, :],
                        xk_r[:, ch, tb0 * 512:(tb0 + 2) * 512])
                xt_tiles[tb0] = xt

            id_sb = consts.tile([128, 128], F32, name="id_sb")
            nc.sync.dma_start(id_sb[:], ident)
            id2_sb = consts.tile([128, 64], BF16, name="id2_sb")
            nc.sync.dma_start(id2_sb[:], ident2)
            ones_sb = consts.tile([128, NKT], F32, name="ones_sb")
            nc.sync.dma_start(ones_sb[:], ones_kv)

            # ---- persistent activations ----
            # kt2x: pair-group pg holds K^T for t-blocks (2pg, 2pg+1) on
            # partition halves [0:64] / [64:128], columns pg*512 + w.
            kt2x = consts.tile([128, TQ], BF16, name="kt2x")
            # qt2x: Q^T duplicated on both partition halves.
            qt2x = consts.tile([128, TQ], BF16, name="qt2x")
            v_all = consts.tile([128, NKT, H + 1], BF16, name="v_all")

            nc.vector.tensor_copy(v_all[:, :, H], ones_sb[:])

            inv_sqrt_h = 1.0 / np.sqrt(np.float32(H))

            def kt_slice(j):
                tb, s = j // 4, j % 4
                half, pg = tb % 2, tb // 2
                return kt2x[64 * half:64 * (half + 1),
                            pg * 512 + s * 128: pg * 512 + (s + 1) * 128]

            def wslice(wname, ch):
                return w_sb[wname][:, ch * H:(ch + 1) * H]

            def proj_pair(tb0, wname, pdst):
                # A/B chunk matmuls adjacent so column halves stream
                # concurrently.
                xa = xt_tiles[tb0][:, :, 0:512]
                xb = xt_tiles[tb0][:, :, 512:1024]
                for ch in range(NCH):
                    nc.tensor.matmul(pdst[0:64, :], wslice(wname, ch),
                                     xa[:, ch, :],
                                     start=(ch == 0), stop=(ch == NCH - 1))
                    nc.tensor.matmul(pdst[64:128, :], wslice(wname, ch),
                                     xb[:, ch, :],
                                     start=(ch == 0), stop=(ch == NCH - 1),
                                     tile_position=(0, 64))
                return pdst

            def emit_projQ(tb0):
                qoff = 0 if tb0 == 4 else 1024
                pq = proj_pair(tb0, "wq",
                               ps_p.tile([128, 512], F32, tag="pp",
                                         name=f"pq{tb0}"))
                for half in range(2):
                    sl = pq[64 * half:64 * (half + 1), :]
                    dst = slice(qoff + half * 512, qoff + (half + 1) * 512)
                    nc.vector.tensor_copy(qt2x[0:64, dst], sl)
                    nc.vector.tensor_copy(qt2x[64:128, dst], sl)

            def emit_projK(tb0):
                pg = tb0 // 2
                pk = proj_pair(tb0, "wk",
                               ps_p.tile([128, 512], F32, tag="pp",
                                         name=f"pk{tb0}"))
                nc.vector.tensor_copy(kt2x[:, pg * 512:(pg + 1) * 512], pk[:])

            def emit_projV(tb0):
                pv = proj_pair(tb0, "wv",
                               ps_p.tile([128, 512], F32, tag="pp",
                                         name=f"pv{tb0}"))
                vt = vtmp_pool.tile([128, 512], BF16, tag="vt",
                                    name=f"vt{tb0}")
                nc.vector.tensor_copy(vt[:], pv[:])
                for half in range(2):
                    for s in range(4):
                        j = 4 * (tb0 + half) + s
                        ptr = ps_p.tile([128, 64], BF16, tag="pp",
                                        name=f"ptr{j}")
                        nc.tensor.transpose(
                            ptr[:],
                            vt[64 * half:64 * (half + 1),
                               s * 128:(s + 1) * 128],
                            id2_sb[64 * half:64 * (half + 1), :])
                        nc.vector.tensor_copy(v_all[:, j, 0:H], ptr[:])

            attn_state = {}

            def attn_begin(qb):
                poa = ps_o.tile([H + 1, 512], F32, tag="poa", name=f"poa{qb}")
                pob = ps_o.tile([H + 1, 512], F32, tag="pob", name=f"pob{qb}")
                attn_state[qb] = dict(poa=poa, pob=pob, queue=[],
                                      jmax=20 + 4 * qb)

            def attn_flush_one(qb):
                st = attn_state[qb]
                js, es2 = st["queue"].pop(0)
                jmax = st["jmax"]
                for idx, j in enumerate(js):
                    cs = slice(idx * 512, (idx + 1) * 512)
                    nc.tensor.matmul(
                        st["poa"][:], v_all[0:64, j, :],
                        es2[0:64, cs],
                        start=(j == 0), stop=(j == jmax - 1),
                        skip_group_check=True)
                    nc.tensor.matmul(
                        st["pob"][:], v_all[64:128, j, :],
                        es2[64:128, cs],
                        start=(j == 0), stop=(j == jmax - 1),
                        tile_position=(64, 0),
                        skip_group_check=True)

            def attn_items(qb, items):
                st = attn_state[qb]
                jmax = st["jmax"]
                diag0 = jmax - 4
                qs_a = qt2x[0:64, qb * 512:(qb + 1) * 512]
                qs_b = qt2x[64:128, qb * 512:(qb + 1) * 512]
                for jA, jB in items:
                    ps = ps_s.tile([128, 1024], F32, tag="ps",
                                   name=f"s{qb}_{jA}")
                    nc.tensor.matmul(ps[:, 0:512], kt_slice(jA),
                                     qs_a, start=True, stop=True)
                    es2 = exps_pool.tile([128, 1024], BF16, tag="es",
                                         name=f"e{qb}_{jA}")
                    if jB is not None:
                        nc.tensor.matmul(ps[:, 512:1024], kt_slice(jB),
                                         qs_b, start=True, stop=True,
                                         tile_position=(64, 0))
                        nc.scalar.activation(es2[:], ps[:],
                                             mybir.ActivationFunctionType.Exp,
                                             scale=float(inv_sqrt_h))
                        if jB >= diag0:
                            d = jB - diag0
                            nc.gpsimd.affine_select(
                                es2[:, 512:1024], es2[:, 512:1024],
                                pattern=[[1, 512]],
                                compare_op=mybir.AluOpType.is_ge,
                                fill=0.0, base=-(128 * d),
                                channel_multiplier=-1)
                        st["queue"].append(((jA, jB), es2))
                    else:
                        nc.scalar.activation(es2[:, 0:512], ps[:, 0:512],
                                             mybir.ActivationFunctionType.Exp,
                                             scale=float(inv_sqrt_h))
                        if jA >= diag0:
                            d = jA - diag0
                            nc.gpsimd.affine_select(
                                es2[:, 0:512], es2[:, 0:512],
                                pattern=[[1, 512]],
                                compare_op=mybir.AluOpType.is_ge,
                                fill=0.0, base=-(128 * d),
                                channel_multiplier=-1)
                        st["queue"].append(((jA,), es2))
                    if len(st["queue"]) > 1:
                        attn_flush_one(qb)

            def attn_flush(qb):
                st = attn_state[qb]
                while st["queue"]:
                    attn_flush_one(qb)

            def attn_final(qb):
                st = attn_state[qb]
                ot = fin_pool.tile([H + 1, 512], F32, tag="ot", name=f"ot{qb}")
                ota = fin_pool.tile([H + 1, 512], F32, tag="ota",
                                    name=f"ota{qb}")
                # po = po_a + po_b, also frees both psum accumulators.
                # (only one PSUM operand legal per DVE op: stage po_a in SBUF)
                nc.vector.tensor_copy(ota[:], st["poa"][:])
                nc.vector.scalar_tensor_tensor(
                    ot[:], st["pob"][:], 1.0, ota[:],
                    op0=mybir.AluOpType.mult, op1=mybir.AluOpType.add)
                ob = fin_pool.tile([128, 4, H], F32, tag="ob", name=f"ob{qb}")
                for s in range(4):
                    ptr = ps_p.tile([128, H + 1], F32, tag="pp",
                                    name=f"otr{qb}_{s}")
                    nc.tensor.transpose(ptr[:], ot[:, s * 128:(s + 1) * 128],
                                        id_sb[:H + 1, :H + 1])
                    rc = fin_pool.tile([128, 1], F32, tag="rc",
                                       name=f"rc{qb}_{s}")
                    nc.vector.reciprocal(rc[:], ptr[:, H:H + 1])
                    nc.vector.tensor_scalar_mul(ob[:, s, :], ptr[:, 0:H], rc[:])
                nc.sync.dma_start(out[qb], ob[:])

            def G(g):
                return [(8 * g + s, 8 * g + 4 + s) for s in range(4)]

            # ---- interleaved emission schedule ----
            # Constraint (program order = dataflow): projK/projQ before the
            # scores that read them; projV before the (lag-1 deferred)
            # O-matmuls that read its v_all tiles.
            emit_projQ(4)
            emit_projK(4)
            emit_projK(0)
            emit_projV(0)
            emit_projV(4)
            attn_begin(0)
            attn_items(0, G(0))
            emit_projK(2)
            emit_projV(2)
            attn_items(0, G(1))
            attn_items(0, [(j, None) for j in range(16, 20)])
            attn_flush(0)
            attn_begin(1)
            attn_items(1, G(0))
            attn_final(0)
            attn_items(1, G(1))
            emit_projQ(6)
            attn_items(1, G(2))
            attn_flush(1)
            emit_projK(6)
            attn_begin(2)
            attn_items(2, G(0))
            attn_final(1)
            attn_items(2, G(1))
            emit_projV(6)
            attn_items(2, G(2))
            attn_items(2, [(j, None) for j in range(24, 28)])
            attn_flush(2)
            attn_begin(3)
            attn_items(3, G(0))
            attn_final(2)
            for g in range(1, 4):
                attn_items(3, G(g))
            attn_flush(3)
            attn_final(3)
    nc.compile()
    _nc_cache["nc"] = nc
    return nc


def _core_inputs(x, Wq, Wk, Wv, core):
    b, h = core // 2, core % 2
    xkm = np.zeros((C, T), dtype=np.float32)
    nk = 2048 * (h + 1)
    xkm[:, T - nk:] = x[b, 0:nk, :].T
    ones = np.zeros((128, NKT), dtype=np.float32)
    ones[:, (T - nk) // 128:] = 1.0
    id2 = np.zeros((128, 64), dtype=np.float32)
    id2[:64] = np.eye(64, dtype=np.float32)
    id2[64:] = np.eye(64, dtype=np.float32)
    bf = ml_dtypes.bfloat16

    def warr(W):
        w = np.asarray(W, dtype=np.float32)
        return np.ascontiguousarray(
            w.reshape(NCH, 128, H).transpose(1, 0, 2).reshape(128, NCH * H)
            .astype(bf))

    return {
        "xk": np.ascontiguousarray(xkm.astype(bf)),
        "wq": warr(Wq),
        "wk": warr(Wk),
        "wv": warr(Wv),
        "ones_kv": ones,
        "ident": np.eye(128, dtype=np.float32),
        "ident2": id2.astype(bf),
    }


def kernel(x, Wq, Wk, Wv):
    x = np.asarray(x, dtype=np.float32)
    nc = build_module()
    in_maps = [_core_inputs(x, Wq, Wk, Wv, c) for c in range(N_CORES)]
    res = run_bass_kernel_spmd(nc, in_maps, core_ids=list(range(N_CORES)))
    out = np.empty((B, T, H), dtype=np.float32)
    for core in range(N_CORES):
        b, h = core // 2, core % 2
        arr = res.results[core]["out"]          # [qb, p, s, h]
        out[b, 2048 * h:2048 * (h + 1), :] = (
            arr.transpose(0, 2, 1, 3).reshape(TQ, H))
    return out


# revision 6
# speedup vs baseline: 1.3666x; 1.3666x over previous
"""Causal single-head attention (B=4, T=4096, C=1024, H=64) on 8 trn2 cores.

Sharding (v3, split-K): 2 cores per batch element. Core parity h takes the
GLOBAL key tiles {4g+h, 4g+2+h : g in 0..7} (every other 128-key tile) for
ALL 8 query blocks of 512, computing flash-style partial numerators and
denominators that the host combines (A+B, then divide). Both cores run one
identical program on 72 (query-block, key-tile) units -- perfectly balanced
causal work with zero padding.

Host xk layout per core: for g in 0..7, 512 columns = global key tiles
[4g+h, 4g+2+h, 4g+1-h, 4g+3-h] (own diag tiles first). Block g's queries
are xk cols [512g, 512(g+1)) (host unpermutes the output); its keys are
"positions" [0, 2(g+1)) = the first 2 column-blocks of groups 0..g.

Perf structure:
  - matmul inputs bf16 (fp32 PSUM, host fp32 normalize)
  - projections col-packed (array column halves, adjacent A/B emission):
    Q = 4 pairs over all 4096 cols; K/V = 4 half-pairs over key cols only
  - scores row-packed: positions (2i, 2i+1) on array row halves ->
    [128,1024] psum pair, one exp (ACT, scale=1/sqrt(H)) per pair
  - O row-packed via split-K: each tile's 128-key contraction split into
    two 64-key halves accumulating into po_a/po_b psum, summed at final
  - causal diagonal (pair i==g) masked post-exp by one DVE multiply with a
    host mask (per-core data, uniform instruction)
  - denominator via ones-column in v_all; no transposes/divide on device:
    out[g] = [65, 512] partials, host combines
  - 4-phase streaming: phase p loads xk quarter p (per-ch 2D DMAs), projects
    Qp/Ks/Vs, runs attention blocks 2p, 2p+1; proj units of phase p+1 are
    interleaved between attention items (which are exp-bound) so the PE
    fills ACT-wait gaps
"""

import numpy as np
import ml_dtypes

import concourse.bass as bass
import concourse.bacc as bacc
import concourse.tile as tile
from concourse import mybir
from concourse.bass_utils import run_bass_kernel_spmd

B, T, C, H = 4, 4096, 1024, 64
N_CORES = 8
NCH = C // 128       # 8 contraction chunks
NG = 8               # query blocks (512 each) per batch
NPOS = 16            # key tile positions per core
F32 = mybir.dt.float32
BF16 = mybir.dt.bfloat16

_nc_cache = {}


def build_module():
    if "nc" in _nc_cache:
        return _nc_cache["nc"]
    nc = bacc.Bacc("TRN2", target_bir_lowering=False, debug=False,
                   num_devices=N_CORES)
    xk = nc.dram_tensor("xk", [C, T], BF16, kind="ExternalInput").ap()
    wq = nc.dram_tensor("wq", [128, NCH * H], BF16, kind="ExternalInput").ap()
    wk = nc.dram_tensor("wk", [128, NCH * H], BF16, kind="ExternalInput").ap()
    wv = nc.dram_tensor("wv", [128, NCH * H], BF16, kind="ExternalInput").ap()
    ident2 = nc.dram_tensor("ident2", [128, 64], BF16,
                            kind="ExternalInput").ap()
    maskd = nc.dram_tensor("maskd", [128, 1024], BF16,
                           kind="ExternalInput").ap()
    # per-block partials: rows 0:64 = O' numerator^T, row 64 = denominator
    out = nc.dram_tensor("out", [NG, H + 1, 512], F32,
                         kind="ExternalOutput").ap()

    with tile.TileContext(nc) as tc:
        with (
            tc.tile_pool(name="consts", bufs=1) as consts,
            tc.tile_pool(name="vtmp", bufs=2) as vtmp_pool,
            tc.tile_pool(name="exps", bufs=6) as exps_pool,
            tc.tile_pool(name="fin", bufs=2) as fin_pool,
            tc.tile_pool(name="ps_s", bufs=2, space="PSUM") as ps_s,
            tc.tile_pool(name="ps_o", bufs=1, space="PSUM") as ps_o,
            tc.tile_pool(name="ps_p", bufs=2, space="PSUM") as ps_p,
        ):
            xk_r = xk.rearrange("(ch p) t -> p ch t", p=128)

            # ---- consts ----
            w_sb = {}
            for name, ap in (("wq", wq), ("wk", wk), ("wv", wv)):
                t = consts.tile([128, NCH * H], BF16, name=f"{name}_sb")
                nc.sync.dma_start(t[:], ap)
                w_sb[name] = t
            id2_sb = consts.tile([128, 64], BF16, name="id2_sb")
            nc.sync.dma_start(id2_sb[:], ident2)
            mask_sb = consts.tile([128, 1024], BF16, name="mask_sb")
            nc.sync.dma_start(mask_sb[:], maskd)

            # ---- x in SBUF: [p, ch, group(8), sub(4), 128] ----
            xt = consts.tile([128, NCH, NG, 4, 128], BF16, name="xt")
            # quarter-major, ch-minor 2D DMAs; spread early ones off sync
            trig = {0: [nc.scalar, nc.gpsimd, nc.sync, nc.scalar,
                        nc.gpsimd, nc.sync, nc.scalar, nc.gpsimd]}
            for quarter in range(4):
                engs = trig.get(quarter, [nc.sync] * NCH)
                for ch in range(NCH):
                    engs[ch].dma_start(
                        xt[:, ch, 2 * quarter:2 * quarter + 2, :, :],
                        xk_r[:, ch, 1024 * quarter:1024 * (quarter + 1)])

            # ---- persistent activations ----
            # kt2x: position 2i+par at [64*par:(par+1)*64, 128i:128(i+1)]
            kt2x = consts.tile([128, 8 * 128], BF16, name="kt2x")
            # qt2x: Q^T in xk column order, duplicated on partition halves
            qt2x = consts.tile([128, T], BF16, name="qt2x")
            v_all = consts.tile([128, NPOS, H + 1], BF16, name="v_all")
            nc.vector.memset(v_all[:, :, H], 1.0)

            inv_sqrt_h = 1.0 / np.sqrt(np.float32(H))

            def wslice(wname, ch):
                return w_sb[wname][:, ch * H:(ch + 1) * H]

            # ---------- projection units (generators of emission thunks) ---
            def gen_projQ(qp):
                """Q over groups (2qp, 2qp+1) -> qt2x cols [1024qp,+1024)."""
                pq = ps_p.tile([128, 512], F32, tag="pp", name=f"pq{qp}")

                def unit(ch):
                    nc.tensor.matmul(pq[0:64, :], wslice("wq", ch),
                                     xt[:, ch, 2 * qp, :, :],
                                     start=(ch == 0), stop=(ch == NCH - 1))
                    nc.tensor.matmul(pq[64:128, :], wslice("wq", ch),
                                     xt[:, ch, 2 * qp + 1, :, :],
                                     start=(ch == 0), stop=(ch == NCH - 1),
                                     tile_position=(0, 64))

                def fin():
                    for half in range(2):
                        sl = pq[64 * half:64 * (half + 1), :]
                        dst = slice(1024 * qp + half * 512,
                                    1024 * qp + (half + 1) * 512)
                        nc.vector.tensor_copy(qt2x[0:64, dst], sl)
                        nc.vector.tensor_copy(qt2x[64:128, dst], sl)

                return [(lambda ch=ch: unit(ch)) for ch in range(NCH)] + [fin]

            def gen_projK(ku):
                """K for positions (4ku..4ku+3) = groups (2ku, 2ku+1)."""
                pk = ps_p.tile([128, 256], F32, tag="pp", name=f"pk{ku}")

                def unit(ch):
                    nc.tensor.matmul(pk[0:64, :], wslice("wk", ch),
                                     xt[:, ch, 2 * ku:2 * ku + 2, 0, :],
                                     start=(ch == 0), stop=(ch == NCH - 1))
                    nc.tensor.matmul(pk[64:128, :], wslice("wk", ch),
                                     xt[:, ch, 2 * ku:2 * ku + 2, 1, :],
                                     start=(ch == 0), stop=(ch == NCH - 1),
                                     tile_position=(0, 64))

                def fin():
                    cs = slice(256 * ku, 256 * (ku + 1))
                    nc.vector.tensor_copy(kt2x[0:64, cs], pk[0:64, :])
                    nc.vector.tensor_copy(kt2x[64:128, cs], pk[64:128, :])

                return [(lambda ch=ch: unit(ch)) for ch in range(NCH)] + [fin]

            def gen_projV(ku):
                pv = ps_p.tile([128, 256], F32, tag="pp", name=f"pv{ku}")

                def unit(ch):
                    nc.tensor.matmul(pv[0:64, :], wslice("wv", ch),
                                     xt[:, ch, 2 * ku:2 * ku + 2, 0, :],
                                     start=(ch == 0), stop=(ch == NCH - 1))
                    nc.tensor.matmul(pv[64:128, :], wslice("wv", ch),
                                     xt[:, ch, 2 * ku:2 * ku + 2, 1, :],
                                     start=(ch == 0), stop=(ch == NCH - 1),
                                     tile_position=(0, 64))

                def fin():
                    vt = vtmp_pool.tile([128, 256], BF16, tag="vt",
                                        name=f"vt{ku}")
                    nc.vector.tensor_copy(vt[:], pv[:])
                    # partitions 0:64 hold positions (4ku, 4ku+2)? no:
                    # half0 = sub 0 of groups (2ku, 2ku+1) = positions
                    # (4ku, 4ku+2); half1 = sub 1 = (4ku+1, 4ku+3)
                    for half in range(2):
                        for t in range(2):
                            p = 4 * ku + 2 * t + half
                            ptr = ps_p.tile([128, 64], BF16, tag="pp",
                                            name=f"ptr{p}")
                            nc.tensor.transpose(
                                ptr[:],
                                vt[64 * half:64 * (half + 1),
                                   t * 128:(t + 1) * 128],
                                id2_sb[64 * half:64 * (half + 1), :])
                            nc.vector.tensor_copy(v_all[:, p, 0:H], ptr[:])

                return [(lambda ch=ch: unit(ch)) for ch in range(NCH)] + [fin]

            # ---------- attention ----------
            attn_state = {}

            def attn_begin(g):
                poa = ps_o.tile([H + 1, 512], F32, tag="poa", name=f"poa{g}")
                pob = ps_o.tile([H + 1, 512], F32, tag="pob", name=f"pob{g}")
                attn_state[g] = dict(poa=poa, pob=pob, queue=[])

            def attn_flush_one(g):
                st = attn_state[g]
                i, es2 = st["queue"].pop(0)
                for t in range(2):
                    p = 2 * i + t
                    cs = slice(t * 512, (t + 1) * 512)
                    st_fl = (i == 0 and t == 0)
                    sp_fl = (i == g and t == 1)
                    nc.tensor.matmul(
                        st["poa"][:], v_all[0:64, p, :], es2[0:64, cs],
                        start=st_fl, stop=sp_fl, skip_group_check=True)
                    nc.tensor.matmul(
                        st["pob"][:], v_all[64:128, p, :], es2[64:128, cs],
                        start=st_fl, stop=sp_fl, tile_position=(64, 0),
                        skip_group_check=True)

            def attn_item(g, i):
                st = attn_state[g]
                qs_a = qt2x[0:64, g * 512:(g + 1) * 512]
                qs_b = qt2x[64:128, g * 512:(g + 1) * 512]
                ps = ps_s.tile([128, 1024], F32, tag="ps", name=f"s{g}_{i}")
                nc.tensor.matmul(ps[:, 0:512],
                                 kt2x[0:64, 128 * i:128 * (i + 1)],
                                 qs_a, start=True, stop=True)
                nc.tensor.matmul(ps[:, 512:1024],
                                 kt2x[64:128, 128 * i:128 * (i + 1)],
                                 qs_b, start=True, stop=True,
                                 tile_position=(64, 0))
                es2 = exps_pool.tile([128, 1024], BF16, tag="es",
                                     name=f"e{g}_{i}")
                nc.scalar.activation(es2[:], ps[:],
                                     mybir.ActivationFunctionType.Exp,
                                     scale=float(inv_sqrt_h))
                if i == g:
                    # diagonal pair: zero the causally-invalid entries
                    nc.vector.scalar_tensor_tensor(
                        es2[:], es2[:], 1.0, mask_sb[:],
                        op0=mybir.AluOpType.mult, op1=mybir.AluOpType.mult)
                st["queue"].append((i, es2))
                if len(st["queue"]) > 1:
                    attn_flush_one(g)

            def attn_end(g):
                st = attn_state[g]
                while st["queue"]:
                    attn_flush_one(g)
                ota = fin_pool.tile([H + 1, 512], F32, tag="ota",
                                    name=f"ota{g}")
                ot = fin_pool.tile([H + 1, 512], F32, tag="ot", name=f"ot{g}")
                nc.vector.tensor_copy(ota[:], st["poa"][:])
                nc.vector.scalar_tensor_tensor(
                    ot[:], st["pob"][:], 1.0, ota[:],
                    op0=mybir.AluOpType.mult, op1=mybir.AluOpType.add)
                nc.sync.dma_start(out[g], ot[:])

            # ---------- interleaved emission ----------
            def phase_proj_units(ph):
                u = []
                u += gen_projQ(ph)
                u += gen_projK(ph)
                u += gen_projV(ph)
                return u

            # phase 0 proj emitted solid (nothing else to overlap with)
            for f in phase_proj_units(0):
                f()

            for ph in range(4):
                # attention items of phase ph, interleaving proj of ph+1
                nxt = phase_proj_units(ph + 1) if ph < 3 else []
                items = []
                for g in (2 * ph, 2 * ph + 1):
                    items.append(("begin", g))
                    for i in range(g + 1):
                        items.append(("item", g, i))
                    items.append(("end", g))
                n_items = sum(1 for it in items if it[0] == "item")
                per = (len(nxt) + n_items - 1) // n_items if n_items else 0
                k = 0
                for it in items:
                    if it[0] == "begin":
                        attn_begin(it[1])
                    elif it[0] == "end":
                        attn_end(it[1])
                    else:
                        attn_item(it[1], it[2])
                        for _ in range(per):
                            if k < len(nxt):
                                nxt[k]()
                                k += 1
                while k < len(nxt):
                    nxt[k]()
                    k += 1
    nc.compile()
    _nc_cache["nc"] = nc
    return nc


def _sub_order(h):
    return [h, 2 + h, 1 - h, 3 - h]


def _core_inputs(x, Wq, Wk, Wv, core):
    b, h = core // 2, core % 2
    sub = _sub_order(h)
    xkm = np.empty((C, T), dtype=np.float32)
    xb = np.asarray(x[b], dtype=np.float32)  # [T, C]
    for g in range(NG):
        for a, s in enumerate(sub):
            tlo = 128 * (4 * g + s)
            xkm[:, 512 * g + 128 * a: 512 * g + 128 * (a + 1)] = \
                xb[tlo:tlo + 128, :].T
    id2 = np.zeros((128, 64), dtype=np.float32)
    id2[:64] = np.eye(64, dtype=np.float32)
    id2[64:] = np.eye(64, dtype=np.float32)
    # diagonal-pair mask: cols [0,512) vs own tile s=h; [512,1024) vs s=2+h
    k = np.arange(128)[:, None]
    qcol = np.arange(512)[None, :]
    qoff = 128 * np.array(sub)[qcol // 128] + qcol % 128
    m0 = (qoff >= 128 * h + k)
    m1 = (qoff >= 128 * (2 + h) + k)
    mask = np.concatenate([m0, m1], axis=1).astype(np.float32)
    bf = ml_dtypes.bfloat16

    def warr(W):
        w = np.asarray(W, dtype=np.float32)
        return np.ascontiguousarray(
            w.reshape(NCH, 128, H).transpose(1, 0, 2).reshape(128, NCH * H)
            .astype(bf))

    return {
        "xk": np.ascontiguousarray(xkm.astype(bf)),
        "wq": warr(Wq),
        "wk": warr(Wk),
        "wv": warr(Wv),
        "ident2": id2.astype(bf),
        "maskd": np.ascontiguousarray(mask.astype(bf)),
    }


def kernel(x, Wq, Wk, Wv):
    x = np.asarray(x, dtype=np.float32)
    nc = build_module()
    in_maps = [_core_inputs(x, Wq, Wk, Wv, c) for c in range(N_CORES)]
    res = run_bass_kernel_spmd(nc, in_maps, core_ids=list(range(N_CORES)))
    out = np.empty((B, T, H), dtype=np.float32)
    inv = [np.argsort(_sub_order(h)) for h in range(2)]
    for b in range(B):
        pa = res.results[2 * b]["out"].astype(np.float64)      # [8, 65, 512]
        pb = res.results[2 * b + 1]["out"].astype(np.float64)
        # unpermute each core's query columns to global order, then combine
        pa = pa.reshape(NG, H + 1, 4, 128)[:, :, inv[0], :]
        pb = pb.reshape(NG, H + 1, 4, 128)[:, :, inv[1], :]
        num = pa[:, :H] + pb[:, :H]                  # [8, 64, 4, 128]
        den = pa[:, H] + pb[:, H]                    # [8, 4, 128]
        o = num / den[:, None, :, :]                 # [8, 64, 4, 128]
        out[b] = (o.transpose(0, 2, 3, 1)            # [8, 4, 128, 64]
                  .reshape(T, H).astype(np.float32))
    return out
